# revision 37
# baseline (speedup 1.0000x reference)
"""Differential attention (B=1, N=2048, C=1024, H=16) on 8 Trainium2 NeuronCores.

Sharding: tensor-parallel over heads. Each core owns 2 heads: it computes the
QKV projection for its heads only, runs RoPE + the two softmaxes + PV locally,
then the per-head attention outputs (in transposed [dv, n] layout) are
AllGathered chunk-by-chunk (4 x 256KB/rank, far cheaper than the 8MB
all-reduce alternative and overlapped with the attention tail) and each core
computes a 128-column slice of the output projection.

All matmuls run in bf16 with fp32 PSUM accumulation; softmax statistics are
kept in fp32. Softmax skips max-subtraction: logits are q.k/8 with q,k ~ N(0,1)
so |logit| < ~7 and exp() is comfortably inside the fp32 range. Softmax
denominators are computed on the Tensor engine as ones^T @ E accumulated over
key tiles (a [1,512] PSUM row), avoiding any partition-axis reduction.
"""

import os
import sys

import numpy as np
import ml_dtypes

for _p in ("/opt/trn_rl_repo", os.path.expanduser("~/.axon_site/_ro/trn_rl_repo")):
    if os.path.isdir(_p) and _p not in sys.path:
        sys.path.insert(0, _p)

import concourse.bass as bass  # noqa: E402
import concourse.tile as tile  # noqa: E402
from concourse import bacc, bass_isa, mybir  # noqa: E402
from concourse.alu_op_type import AluOpType  # noqa: E402
from concourse.bass_utils import run_bass_kernel_spmd  # noqa: E402

BF16 = ml_dtypes.bfloat16
B, N, C = 1, 2048, 1024
H = 16
HD = 64  # head dim of each rope/attn half
DV = 2 * HD  # value dim per head (128)
NCORES = 8
HPC = H // NCORES  # heads per core = 2
P = 128
KT = C // P  # 8 contraction tiles for QKV
JT = N // P  # 16 key-position tiles
NCH = 4  # 512-wide chunks of the sequence
CH = N // NCH  # 512
SCALE = HD**-0.5

_PROG = None


def _build_program(collective=True):
    dt = mybir.dt
    f32 = dt.float32
    bf = dt.bfloat16
    Exp = mybir.ActivationFunctionType.Exp

    nc = bacc.Bacc(
        "TRN2", target_bir_lowering=False, debug=False, num_devices=NCORES
    )

    xT = nc.dram_tensor("xT", [C, N], bf, kind="ExternalInput")
    wqk = nc.dram_tensor("wqk", [C, 4 * P], bf, kind="ExternalInput")
    wv = nc.dram_tensor("wv", [C, HPC * DV], bf, kind="ExternalInput")
    wpj = nc.dram_tensor("wpj", [2 * C, P], bf, kind="ExternalInput")
    cosT = nc.dram_tensor("cosT", [P, N], bf, kind="ExternalInput")
    sinT = nc.dram_tensor("sinT", [P, N], bf, kind="ExternalInput")
    nlam = nc.dram_tensor("nlam", [1, 1], f32, kind="ExternalInput")
    yT = nc.dram_tensor("yT", [P, N], f32, kind="ExternalOutput")

    with tile.TileContext(nc) as tc:
        with tc.tile_pool(name="const", bufs=1) as const, tc.tile_pool(
            name="dram", bufs=1, space="DRAM"
        ) as dram:
            # ---- persistent SBUF tensors ----
            # x and wqk split per (k-tile, chunk) so the first QKV matmul
            # only waits for ~256KB of DMA, not the whole 6MB input set.
            wqk_sb = [None] * KT
            xsb = [[None] * NCH for _ in range(KT)]

            def load_x(k, c):
                t = const.tile([P, CH], bf, tag=f"xsb{k}_{c}", name=f"xsb{k}_{c}")
                nc.sync.dma_start(
                    t, xT.ap()[k * P : (k + 1) * P, c * CH : (c + 1) * CH]
                )
                xsb[k][c] = t

            def load_x_chunk(c):
                for k in range(KT):
                    load_x(k, c)

            for k in range(KT):
                t = const.tile([P, 4 * P], bf, tag=f"wqk{k}", name=f"wqk{k}")
                nc.sync.dma_start(t, wqk.ap()[k * P : (k + 1) * P, :])
                wqk_sb[k] = t
                load_x(k, 0)
            cos_sb = const.tile([P, N], bf, tag="cos")
            nc.sync.dma_start(cos_sb, cosT.ap())
            sin_sb = const.tile([P, N], bf, tag="sin")
            nc.sync.dma_start(sin_sb, sinT.ap())
            load_x_chunk(1)
            wv_sb = []
            for k in range(KT):
                t = const.tile([P, HPC * DV], bf, tag=f"wv{k}", name=f"wv{k}")
                nc.sync.dma_start(t, wv.ap()[k * P : (k + 1) * P, :])
                wv_sb.append(t)
            load_x_chunk(2)
            load_x_chunk(3)
            nlam_sb = const.tile([1, 1], f32, tag="nlam")
            nc.sync.dma_start(nlam_sb, nlam.ap())
            nlam_bc = const.tile([P, 1], f32, tag="nlambc")
            nc.gpsimd.partition_broadcast(nlam_bc, nlam_sb)
            wpj_sb = []
            for k in range(JT):
                t = const.tile([P, P], bf, tag=f"wpj{k}", name=f"wpj{k}")
                nc.sync.dma_start(t, wpj.ap()[k * P : (k + 1) * P, :])
                wpj_sb.append(t)

            # rope'd projections, [d(2 heads stacked), n], one tile per chunk
            def chunk_tiles(nm):
                return [
                    const.tile([P, CH], bf, tag=f"{nm}{c}", name=f"{nm}{c}")
                    for c in range(NCH)
                ]

            q1c, q2c = chunk_tiles("q1c"), chunk_tiles("q2c")
            k1c, k2c = chunk_tiles("k1c"), chunk_tiles("k2c")
            # values in [j, dv] layout, per head, split into 4 j-groups so
            # PV can start before the whole V projection has finished
            vsb = [
                [
                    const.tile([P, 4, DV], bf, tag=f"vsb{h}_{g}", name=f"vsb{h}_{g}")
                    for g in range(4)
                ]
                for h in range(HPC)
            ]

            # per-chunk AllGather buffers (overlap collectives w/ compute)
            ag_in = [
                dram.tile([HPC * DV, CH], bf, name=f"ag_in{i}") for i in range(NCH)
            ]
            ag_out = [
                dram.tile([2 * C, CH], bf, addr_space="Shared", name=f"ag_out{i}")
                for i in range(NCH)
            ]
            # final chunk gathered per-head so the h=0 half overlaps unit 7
            ag_out3h = [
                dram.tile([C, CH], bf, addr_space="Shared", name=f"ag_out3h{i}")
                for i in range(HPC)
            ]

            # ---- phase 1 prologue: keys, q-chunk 0, first V j-group ----
            def emit_v(nb, pool, tag):
                c, off = nb // 4, (nb % 4) * P
                pvs = pool.tile([P, HPC * DV], f32, tag=tag, name=f"vstrip{nb}")
                for k in range(KT):
                    nc.tensor.matmul(
                        pvs,
                        lhsT=xsb[k][c][:, off : off + P],
                        rhs=wv_sb[k],
                        start=(k == 0),
                        stop=(k == KT - 1),
                    )
                for h in range(HPC):
                    nc.scalar.copy(
                        vsb[h][nb // 4][:, nb % 4, :], pvs[:, h * DV : (h + 1) * DV]
                    )

            def emit_qk_strip(m, nci, pool, tag):
                pt = pool.tile([P, CH], f32, tag=tag, name=f"qkstrip{m}_{nci}")
                for k in range(KT):
                    nc.tensor.matmul(
                        pt,
                        lhsT=wqk_sb[k][:, m * P : (m + 1) * P],
                        rhs=xsb[k][nci],
                        start=(k == 0),
                        stop=(k == KT - 1),
                    )
                return pt

            def emit_rope(s1t, s2t, o1, o2, nci, pool):
                ns = slice(nci * CH, (nci + 1) * CH)
                a = pool.tile([P, CH], f32, tag="ropetmp", name="rt_a")
                b = pool.tile([P, CH], f32, tag="ropetmp", name="rt_b")
                nc.vector.tensor_tensor(a, s1t, cos_sb[:, ns], AluOpType.mult)
                nc.vector.tensor_tensor(b, s2t, sin_sb[:, ns], AluOpType.mult)
                nc.vector.tensor_tensor(o1, a, b, AluOpType.subtract)
                a2 = pool.tile([P, CH], f32, tag="ropetmp", name="rt_a2")
                b2 = pool.tile([P, CH], f32, tag="ropetmp", name="rt_b2")
                nc.vector.tensor_tensor(a2, s2t, cos_sb[:, ns], AluOpType.mult)
                nc.vector.tensor_tensor(b2, s1t, sin_sb[:, ns], AluOpType.mult)
                nc.vector.tensor_tensor(o2, a2, b2, AluOpType.add)

            rope_cm = tc.tile_pool(name="rope", bufs=8)
            rope_t = rope_cm.__enter__()
            with tc.tile_pool(name="p_pro", bufs=6, space="PSUM") as p_pro, \
                 tc.tile_pool(name="p_prov", bufs=2, space="PSUM") as p_prov:
                for nci in range(NCH):
                    sk1 = emit_qk_strip(2, nci, p_pro, "prostrip")
                    sk2 = emit_qk_strip(3, nci, p_pro, "prostrip")
                    emit_rope(sk1, sk2, k1c[nci], k2c[nci], nci, rope_t)
                sq1 = emit_qk_strip(0, 0, p_pro, "prostrip")
                sq2 = emit_qk_strip(1, 0, p_pro, "prostrip")
                emit_rope(sq1, sq2, q1c[0], q2c[0], 0, rope_t)
                for nb in range(4):
                    emit_v(nb, p_prov, "vpro")

            # ---- phase 2: flat attention pipeline + gathered projection ----
            # Remaining V j-groups and q-chunks 1-3 are injected as filler
            # tasks into the early pipeline slots.
            with tc.tile_pool(name="e", bufs=2) as e_pool, \
                 tc.tile_pool(name="red", bufs=2) as red, \
                 tc.tile_pool(name="comb", bufs=2) as comb, \
                 tc.tile_pool(name="agst", bufs=2) as agst, \
                 tc.tile_pool(name="prhs", bufs=4) as prhs, \
                 tc.tile_pool(name="yst", bufs=1) as yst, \
                 tc.tile_pool(name="p_s", bufs=2, space="PSUM") as p_s, \
                 tc.tile_pool(name="p_pv", bufs=3, space="PSUM") as p_pv, \
                 tc.tile_pool(name="p_y", bufs=1, space="PSUM") as p_y:

                def emit_allgather(ic):
                    if collective:
                        nc.gpsimd.collective_compute(
                            "AllGather",
                            AluOpType.bypass,
                            replica_groups=[list(range(NCORES))],
                            ins=[ag_in[ic][:, :]],
                            outs=[ag_out[ic][:, :]],
                        )
                    else:
                        # timing-only stand-in (single-core TimelineSim)
                        nc.sync.dma_start(ag_out[ic][: HPC * DV, :], ag_in[ic][:, :])

                proj_state = {}

                def emit_proj_dma(ic):
                    rchs = []
                    for g in range(4):
                        rch = prhs.tile(
                            [P, 4, CH], bf, tag="rch", name=f"rch{ic}_{g}", bufs=4
                        )
                        nc.sync.dma_start(
                            rch,
                            ag_out[ic][g * 4 * P : (g + 1) * 4 * P, :].rearrange(
                                "(t p) n -> p t n", p=P
                            ),
                        )
                        rchs.append(rch)
                    proj_state[ic] = rchs

                def emit_proj_mm(ic):
                    py = p_y.tile([P, CH], f32, tag="y", name=f"py{ic}")
                    rchs = proj_state.pop(ic)
                    for g in range(4):
                        for t in range(4):
                            kt = g * 4 + t
                            nc.tensor.matmul(
                                py, lhsT=wpj_sb[kt], rhs=rchs[g][:, t, :],
                                start=(kt == 0), stop=(kt == JT - 1),
                            )
                    ysb = yst.tile([P, CH], f32, tag="ysb", name=f"ysb{ic}")
                    nc.scalar.copy(ysb, py)
                    nc.sync.dma_start(yT.ap()[:, ic * CH : (ic + 1) * CH], ysb)

                py3 = [None]

                def emit_proj3_head(hh):
                    # ag_out3h[hh] rows r*128 : (r+1)*128 hold head (2r + hh),
                    # i.e. global kt = 2r + hh
                    if py3[0] is None:
                        py3[0] = p_y.tile([P, CH], f32, tag="y", name="py3")
                    py = py3[0]
                    rchs = []
                    for g2 in range(2):
                        rch = prhs.tile(
                            [P, 4, CH], bf, tag="rch", name=f"rch3{hh}_{g2}", bufs=4
                        )
                        nc.sync.dma_start(
                            rch,
                            ag_out3h[hh][g2 * 4 * P : (g2 + 1) * 4 * P, :].rearrange(
                                "(t p) n -> p t n", p=P
                            ),
                        )
                        rchs.append(rch)
                    for g2 in range(2):
                        for t in range(4):
                            r = g2 * 4 + t
                            kt = 2 * r + hh
                            nc.tensor.matmul(
                                py, lhsT=wpj_sb[kt], rhs=rchs[g2][:, t, :],
                                start=(hh == 0 and r == 0),
                                stop=(hh == 1 and r == 7),
                                skip_group_check=True,
                            )
                    if hh == 1:
                        ysb = yst.tile([P, CH], f32, tag="ysb", name="ysb3")
                        nc.scalar.copy(ysb, py)
                        nc.sync.dma_start(
                            yT.ap()[:, (NCH - 1) * CH : NCH * CH], ysb
                        )

                UNITS = [(ic, hh) for ic in range(NCH) for hh in range(HPC)]
                NU = len(UNITS)
                NPAIR = JT // 2
                st = [None] * NU

                def unit_alloc(u):
                    st[u] = dict(
                        e1=e_pool.tile([P, JT, CH], bf, tag="e1", name=f"e1_{u}"),
                        e2=e_pool.tile([P, JT, CH], bf, tag="e2", name=f"e2_{u}"),
                        pv1=p_pv.tile([P, CH], f32, tag="pv", name=f"pv1_{u}"),
                        pv2=p_pv.tile([P, CH], f32, tag="pv", name=f"pv2_{u}"),
                        acc1=red.tile([P, CH], f32, tag="acc1", name=f"acc1_{u}"),
                        # bf16: walrus rejects mixed-dtype TensorTensor on
                        # GPSIMD; the 128-partial average washes out rounding
                        acc2=red.tile([P, CH], bf, tag="acc2", name=f"acc2_{u}"),
                    )

                def emit_s(u, p):
                    ic, hh = UNITS[u]
                    hs = slice(HD * hh, HD * (hh + 1))
                    jb = 2 * p
                    c, o0, o1 = jb // 4, (jb % 4) * P, (jb % 4 + 1) * P
                    e1, e2 = st[u]["e1"], st[u]["e2"]
                    s1 = p_s.tile([P, 2, CH], f32, tag="s", name="s1t")
                    nc.tensor.matmul(
                        s1[:, 0, :], lhsT=k1c[c][hs, o0 : o0 + P],
                        rhs=q1c[ic][hs, :], start=True, stop=True,
                    )
                    nc.tensor.matmul(
                        s1[:, 1, :], lhsT=k1c[c][hs, o1 : o1 + P],
                        rhs=q1c[ic][hs, :], start=True, stop=True,
                    )
                    nc.scalar.activation(e1[:, jb : jb + 2, :], s1, Exp, scale=SCALE)
                    s2 = p_s.tile([P, 2, CH], f32, tag="s", name="s2t")
                    nc.tensor.matmul(
                        s2[:, 0, :], lhsT=k2c[c][hs, o0 : o0 + P],
                        rhs=q2c[ic][hs, :], start=True, stop=True,
                    )
                    nc.tensor.matmul(
                        s2[:, 1, :], lhsT=k2c[c][hs, o1 : o1 + P],
                        rhs=q2c[ic][hs, :], start=True, stop=True,
                    )
                    nc.scalar.activation(e2[:, jb : jb + 2, :], s2, Exp, scale=SCALE)

                def emit_acc(u, p):
                    # softmax denominators: E1 strips summed on DVE, E2 split
                    # DVE/GPSIMD (Pool adds are ~1.9x slower than DVE), one
                    # pair behind the exps
                    e1, e2 = st[u]["e1"], st[u]["e2"]
                    acc1, acc2 = st[u]["acc1"], st[u]["acc2"]
                    jb = 2 * p
                    e2_eng = nc.vector if (p in (0, 1, 2) or u == NU - 1) else nc.gpsimd
                    if p == 0:
                        nc.vector.tensor_tensor(
                            acc1, e1[:, 0, :], e1[:, 1, :], AluOpType.add
                        )
                        e2_eng.tensor_tensor(
                            acc2, e2[:, 0, :], e2[:, 1, :], AluOpType.add
                        )
                    else:
                        for j in (jb, jb + 1):
                            nc.vector.tensor_tensor(
                                acc1, acc1, e1[:, j, :], AluOpType.add
                            )
                            e2_eng.tensor_tensor(
                                acc2, acc2, e2[:, j, :], AluOpType.add
                            )

                def finish_acc(u):
                    acc1, acc2 = st[u]["acc1"], st[u]["acc2"]
                    s1bc = red.tile([P, CH], f32, tag="s1bc", name=f"s1bc{u}", bufs=2)
                    nc.gpsimd.partition_all_reduce(
                        s1bc, acc1, 128, bass_isa.ReduceOp.add
                    )
                    s2bc = red.tile([P, CH], f32, tag="s2bc", name=f"s2bc{u}", bufs=2)
                    nc.gpsimd.partition_all_reduce(
                        s2bc, acc2, 128, bass_isa.ReduceOp.add
                    )
                    st[u]["s1bc"], st[u]["s2bc"] = s1bc, s2bc

                def finish_recip(u):
                    if "r1" in st[u]:
                        return
                    # one slot after the partition reduce so the reciprocal
                    # doesn't head-of-line-block DVE while Pool finishes
                    r1 = red.tile([P, CH], f32, tag="r1", name=f"r1_{u}", bufs=1)
                    nc.vector.reciprocal(r1, st[u]["s1bc"])
                    r2 = red.tile([P, CH], f32, tag="r2", name=f"r2_{u}", bufs=1)
                    nc.vector.reciprocal(r2, st[u]["s2bc"])
                    st[u]["r1"], st[u]["r2"] = r1, r2

                def emit_pv(u, p):
                    ic, hh = UNITS[u]
                    e1, e2 = st[u]["e1"], st[u]["e2"]
                    pv1, pv2 = st[u]["pv1"], st[u]["pv2"]
                    for j in (2 * p, 2 * p + 1):
                        vt = vsb[hh][j // 4][:, j % 4, :]
                        nc.tensor.matmul(
                            pv1, lhsT=vt, rhs=e1[:, j, :],
                            start=(j == 0), stop=(j == JT - 1),
                            skip_group_check=True,
                        )
                        nc.tensor.matmul(
                            pv2, lhsT=vt, rhs=e2[:, j, :],
                            start=(j == 0), stop=(j == JT - 1),
                            skip_group_check=True,
                        )

                def finish_unit(u):
                    ic, hh = UNITS[u]
                    pv1, pv2 = st[u]["pv1"], st[u]["pv2"]
                    r1, r2 = st[u]["r1"], st[u]["r2"]
                    t1 = comb.tile([P, CH], f32, tag="t1", name=f"t1_{u}")
                    nc.vector.tensor_tensor(t1, pv1, r1, AluOpType.mult)
                    t2 = comb.tile([P, CH], f32, tag="t2", name=f"t2_{u}")
                    nc.vector.scalar_tensor_tensor(
                        t2, pv2, nlam_bc, r2, AluOpType.mult, AluOpType.mult
                    )
                    oc = agst.tile([P, CH], bf, tag="oc", name=f"oc{u}")
                    nc.vector.tensor_tensor(oc, t1, t2, AluOpType.add)
                    nc.sync.dma_start(ag_in[ic][hh * P : (hh + 1) * P, :], oc)
                    if ic == NCH - 1:
                        # per-head gather: h=0 fires a whole unit earlier
                        if collective:
                            nc.gpsimd.collective_compute(
                                "AllGather",
                                AluOpType.bypass,
                                replica_groups=[list(range(NCORES))],
                                ins=[ag_in[ic][hh * P : (hh + 1) * P, :]],
                                outs=[ag_out3h[hh][:, :]],
                            )
                        else:
                            nc.sync.dma_start(
                                ag_out3h[hh][:P, :], ag_in[ic][hh * P : (hh + 1) * P, :]
                            )
                    elif hh == HPC - 1:
                        emit_allgather(ic)

                # filler tasks for early slots: 2 V j-blocks per slot, then
                # the remaining q-chunk projections
                def v_task(nbs):
                    emit_v(nbs, p_pv, "pv")
                    emit_v(nbs + 1, p_pv, "pv")

                def q_task(nci):
                    sq1 = emit_qk_strip(0, nci, p_s, "s")
                    sq2 = emit_qk_strip(1, nci, p_s, "s")
                    emit_rope(sq1, sq2, q1c[nci], q2c[nci], nci, rope_t)

                tasks = [
                    lambda: v_task(4),
                    lambda: v_task(6),
                    lambda: q_task(1),
                    lambda: v_task(8),
                    lambda: v_task(10),
                    lambda: q_task(2),
                    lambda: v_task(12),
                    lambda: v_task(14),
                    lambda: q_task(3),
                ]

                ACC_LAG, PV_LAG = 1, 4
                total = NU * NPAIR
                proj_at = {(2 * ic + 3): ic for ic in range(NCH - 1)}
                # final-chunk proj handled via emit_proj3_head
                LAST_PV_LAG = 2  # shrink the exposed epilogue of the last unit

                def pv_lag(pair):
                    return LAST_PV_LAG if pair >= (NU - 1) * NPAIR else PV_LAG

                pv_next = 0
                for g in range(total + PV_LAG + 1):
                    if tasks and g < 9:
                        tasks.pop(0)()
                    if g < total:
                        u, p = divmod(g, NPAIR)
                        if p == 0:
                            unit_alloc(u)
                            if u in proj_at:
                                emit_proj_dma(proj_at[u])
                        if p == 3 and u in proj_at:
                            emit_proj_mm(proj_at[u])
                        emit_s(u, p)
                    ga = g - ACC_LAG
                    if 0 <= ga < total:
                        u, p = divmod(ga, NPAIR)
                        emit_acc(u, p)
                        if p == NPAIR - 1:
                            finish_acc(u)
                    gr = g - ACC_LAG - 1
                    if 0 <= gr < total:
                        u, p = divmod(gr, NPAIR)
                        if p == NPAIR - 1:
                            finish_recip(u)
                    while pv_next < total and pv_next + pv_lag(pv_next) <= g:
                        u, p = divmod(pv_next, NPAIR)
                        emit_pv(u, p)
                        if p == NPAIR - 1:
                            if u == NU - 1:
                                finish_recip(u)
                            finish_unit(u)
                        pv_next += 1
                    if g == total - 3:
                        emit_proj3_head(0)
                emit_proj3_head(1)
            rope_cm.__exit__(None, None, None)

    nc.compile()
    return nc


def _get_prog():
    global _PROG
    if _PROG is None:
        _PROG = _build_program()
    return _PROG


def _prep_in_maps(x, W_qkv, W_proj, lambda_q1, lambda_q2, lambda_k1, lambda_k2):
    x = np.asarray(x, np.float32).reshape(N, C)
    W_qkv = np.asarray(W_qkv, np.float32)
    W_proj = np.asarray(W_proj, np.float32)

    xT = np.ascontiguousarray(x.T).astype(BF16)

    inv_freq = 1.0 / (10000.0 ** (np.arange(0, DV, 2, dtype=np.float32) / DV))
    freqs = np.arange(N, dtype=np.float32)[:, None] * inv_freq[None, :]  # [N, 64]
    cos = np.cos(freqs).astype(np.float32).T  # [64, N]
    sin = np.sin(freqs).astype(np.float32).T
    cosT = np.concatenate([cos, cos], axis=0).astype(BF16)  # [128, N]
    sinT = np.concatenate([sin, sin], axis=0).astype(BF16)

    lam_init = 0.8 - 0.6 * float(np.exp(-0.3 * 0.0))
    lam = (
        float(np.exp(np.sum(lambda_q1.astype(np.float32) * lambda_k1.astype(np.float32))))
        - float(np.exp(np.sum(lambda_q2.astype(np.float32) * lambda_k2.astype(np.float32))))
        + lam_init
    )
    nlam = np.full((1, 1), -lam, dtype=np.float32)

    W_projT = np.ascontiguousarray(W_proj.T)  # [2C, C]

    in_maps = []
    for r in range(NCORES):
        hA, hB = 2 * r, 2 * r + 1
        idx_qk = np.concatenate(
            [
                np.arange(g * C + h * HD, g * C + (h + 1) * HD)
                for g in range(4)
                for h in (hA, hB)
            ]
        )
        idx_v = np.concatenate(
            [
                np.arange(g * C + h * HD, g * C + (h + 1) * HD)
                for h in (hA, hB)
                for g in (4, 5)
            ]
        )
        wqk_r = np.ascontiguousarray(W_qkv[idx_qk, :].T).astype(BF16)  # [C, 512]
        wv_r = np.ascontiguousarray(W_qkv[idx_v, :].T).astype(BF16)  # [C, 256]
        wpj_r = np.ascontiguousarray(W_projT[:, r * P : (r + 1) * P]).astype(BF16)
        in_maps.append(
            dict(
                xT=xT,
                wqk=wqk_r,
                wv=wv_r,
                wpj=wpj_r,
                cosT=cosT,
                sinT=sinT,
                nlam=nlam,
            )
        )
    return in_maps


LAST_EXEC_TIME_NS = None


def kernel(x, W_qkv, W_proj, lambda_q1, lambda_q2, lambda_k1, lambda_k2):
    global LAST_EXEC_TIME_NS
    nc = _get_prog()
    in_maps = _prep_in_maps(
        x, W_qkv, W_proj, lambda_q1, lambda_q2, lambda_k1, lambda_k2
    )
    res = run_bass_kernel_spmd(nc, in_maps, core_ids=list(range(NCORES)))
    LAST_EXEC_TIME_NS = res.exec_time_ns
    yT_full = np.concatenate([res.results[r]["yT"] for r in range(NCORES)], axis=0)
    return np.ascontiguousarray(yT_full.T).reshape(B, N, C).astype(np.float32)


# revision 47
# speedup vs baseline: 1.1109x; 1.1109x over previous
"""Differential attention (B=1, N=2048, C=1024, H=16) on 8 Trainium2 NeuronCores.

Sharding: tensor-parallel over heads. Each core owns 2 heads: it computes the
QKV projection for its heads only, runs RoPE + the two softmaxes + PV locally,
then the per-head attention outputs (in transposed [dv, n] layout) are
AllGathered chunk-by-chunk (4 x 256KB/rank, far cheaper than the 8MB
all-reduce alternative and overlapped with the attention tail) and each core
computes a 128-column slice of the output projection.

All matmuls run in bf16 with fp32 PSUM accumulation; softmax statistics are
kept in fp32. Softmax skips max-subtraction: logits are q.k/8 with q,k ~ N(0,1)
so |logit| < ~7 and exp() is comfortably inside the fp32 range. Softmax
denominators are computed on the Tensor engine as ones^T @ E accumulated over
key tiles (a [1,512] PSUM row), avoiding any partition-axis reduction.
"""

import os
import sys

import numpy as np
import ml_dtypes

for _p in ("/opt/trn_rl_repo", os.path.expanduser("~/.axon_site/_ro/trn_rl_repo")):
    if os.path.isdir(_p) and _p not in sys.path:
        sys.path.insert(0, _p)

import concourse.bass as bass  # noqa: E402
import concourse.tile as tile  # noqa: E402
from concourse import bacc, bass_isa, mybir  # noqa: E402
from concourse.alu_op_type import AluOpType  # noqa: E402
from concourse.bass_utils import run_bass_kernel_spmd  # noqa: E402

BF16 = ml_dtypes.bfloat16
B, N, C = 1, 2048, 1024
H = 16
HD = 64  # head dim of each rope/attn half
DV = 2 * HD  # value dim per head (128)
NCORES = 8
HPC = H // NCORES  # heads per core = 2
P = 128
KT = C // P  # 8 contraction tiles for QKV
JT = N // P  # 16 key-position tiles
NCH = 4  # 512-wide chunks of the sequence
CH = N // NCH  # 512
SCALE = HD**-0.5

_PROG = None


def _build_program(collective=True):
    dt = mybir.dt
    f32 = dt.float32
    bf = dt.bfloat16
    Exp = mybir.ActivationFunctionType.Exp

    nc = bacc.Bacc(
        "TRN2", target_bir_lowering=False, debug=False, num_devices=NCORES
    )

    xT = nc.dram_tensor("xT", [C, N], bf, kind="ExternalInput")
    wqk = nc.dram_tensor("wqk", [C, 4 * P], bf, kind="ExternalInput")
    wv = nc.dram_tensor("wv", [C, HPC * DV], bf, kind="ExternalInput")
    wpj = nc.dram_tensor("wpj", [2 * C, P], bf, kind="ExternalInput")
    cosT = nc.dram_tensor("cosT", [P, N], bf, kind="ExternalInput")
    sinT = nc.dram_tensor("sinT", [P, N], bf, kind="ExternalInput")
    nlam = nc.dram_tensor("nlam", [1, 1], f32, kind="ExternalInput")
    yT = nc.dram_tensor("yT", [P, N], f32, kind="ExternalOutput")

    with tile.TileContext(nc) as tc:
        with tc.tile_pool(name="const", bufs=1) as const, tc.tile_pool(
            name="dram", bufs=1, space="DRAM"
        ) as dram:
            # ---- persistent SBUF tensors ----
            # x and wqk split per (k-tile, chunk) so the first QKV matmul
            # only waits for ~256KB of DMA, not the whole 6MB input set.
            wqk_sb = [None] * KT
            xsb = [[None] * NCH for _ in range(KT)]

            def load_x(k, c):
                t = const.tile([P, CH], bf, tag=f"xsb{k}_{c}", name=f"xsb{k}_{c}")
                nc.sync.dma_start(
                    t, xT.ap()[k * P : (k + 1) * P, c * CH : (c + 1) * CH]
                )
                xsb[k][c] = t

            def load_x_chunk(c):
                for k in range(KT):
                    load_x(k, c)

            for k in range(KT):
                t = const.tile([P, 4 * P], bf, tag=f"wqk{k}", name=f"wqk{k}")
                nc.sync.dma_start(t, wqk.ap()[k * P : (k + 1) * P, :])
                wqk_sb[k] = t
                load_x(k, 0)
            cos_sb = const.tile([P, N], bf, tag="cos")
            nc.sync.dma_start(cos_sb, cosT.ap())
            sin_sb = const.tile([P, N], bf, tag="sin")
            nc.sync.dma_start(sin_sb, sinT.ap())
            load_x_chunk(1)
            wv_sb = []
            for k in range(KT):
                t = const.tile([P, HPC * DV], bf, tag=f"wv{k}", name=f"wv{k}")
                nc.sync.dma_start(t, wv.ap()[k * P : (k + 1) * P, :])
                wv_sb.append(t)
            load_x_chunk(2)
            load_x_chunk(3)
            nlam_sb = const.tile([1, 1], f32, tag="nlam")
            nc.sync.dma_start(nlam_sb, nlam.ap())
            nlam_bc = const.tile([P, 1], f32, tag="nlambc")
            nc.gpsimd.partition_broadcast(nlam_bc, nlam_sb)
            wpj_sb = []
            for k in range(JT):
                t = const.tile([P, P], bf, tag=f"wpj{k}", name=f"wpj{k}")
                nc.sync.dma_start(t, wpj.ap()[k * P : (k + 1) * P, :])
                wpj_sb.append(t)

            # rope'd projections, [d(2 heads stacked), n], one tile per chunk
            def chunk_tiles(nm):
                return [
                    const.tile([P, CH], bf, tag=f"{nm}{c}", name=f"{nm}{c}")
                    for c in range(NCH)
                ]

            q1c, q2c = chunk_tiles("q1c"), chunk_tiles("q2c")
            k1c, k2c = chunk_tiles("k1c"), chunk_tiles("k2c")
            # values in [j, dv] layout, per head, split into 4 j-groups so
            # PV can start before the whole V projection has finished
            vsb = [
                [
                    const.tile([P, 4, DV], bf, tag=f"vsb{h}_{g}", name=f"vsb{h}_{g}")
                    for g in range(4)
                ]
                for h in range(HPC)
            ]

            # per-chunk AllGather buffers (overlap collectives w/ compute)
            ag_in = [
                dram.tile([HPC * DV, CH], bf, name=f"ag_in{i}") for i in range(NCH)
            ]
            ag_out = [
                dram.tile([2 * C, CH], bf, addr_space="Shared", name=f"ag_out{i}")
                for i in range(NCH)
            ]
            # final chunk gathered per-head so the h=0 half overlaps unit 7
            ag_out3h = [
                dram.tile([C, CH], bf, addr_space="Shared", name=f"ag_out3h{i}")
                for i in range(HPC)
            ]

            # ---- phase 1 prologue: keys, q-chunk 0, first V j-group ----
            def emit_v(nb, pool, tag):
                c, off = nb // 4, (nb % 4) * P
                pvs = pool.tile([P, HPC * DV], f32, tag=tag, name=f"vstrip{nb}")
                for k in range(KT):
                    nc.tensor.matmul(
                        pvs,
                        lhsT=xsb[k][c][:, off : off + P],
                        rhs=wv_sb[k],
                        start=(k == 0),
                        stop=(k == KT - 1),
                    )
                for h in range(HPC):
                    nc.scalar.copy(
                        vsb[h][nb // 4][:, nb % 4, :], pvs[:, h * DV : (h + 1) * DV]
                    )

            def emit_qk_strip(m, nci, pool, tag):
                pt = pool.tile([P, CH], f32, tag=tag, name=f"qkstrip{m}_{nci}")
                for k in range(KT):
                    nc.tensor.matmul(
                        pt,
                        lhsT=wqk_sb[k][:, m * P : (m + 1) * P],
                        rhs=xsb[k][nci],
                        start=(k == 0),
                        stop=(k == KT - 1),
                    )
                return pt

            def emit_rope(s1t, s2t, o1, o2, nci, pool):
                # ops ordered so each PSUM strip is read by exactly the first
                # two DVE ops touching it, releasing its slot early
                ns = slice(nci * CH, (nci + 1) * CH)
                a = pool.tile([P, CH], f32, tag="ropetmp", name="rt_a")
                b2 = pool.tile([P, CH], f32, tag="ropetmp", name="rt_b2")
                nc.vector.tensor_tensor(a, s1t, cos_sb[:, ns], AluOpType.mult)
                nc.vector.tensor_tensor(b2, s1t, sin_sb[:, ns], AluOpType.mult)
                b = pool.tile([P, CH], f32, tag="ropetmp", name="rt_b")
                a2 = pool.tile([P, CH], f32, tag="ropetmp", name="rt_a2")
                nc.vector.tensor_tensor(b, s2t, sin_sb[:, ns], AluOpType.mult)
                nc.vector.tensor_tensor(a2, s2t, cos_sb[:, ns], AluOpType.mult)
                nc.vector.tensor_tensor(o1, a, b, AluOpType.subtract)
                nc.vector.tensor_tensor(o2, a2, b2, AluOpType.add)

            rope_cm = tc.tile_pool(name="rope", bufs=8)
            rope_t = rope_cm.__enter__()
            with tc.tile_pool(name="p_pro", bufs=6, space="PSUM") as p_pro, \
                 tc.tile_pool(name="p_prov", bufs=2, space="PSUM") as p_prov:
                # unit-0 pair p only needs key chunk p//2: rope k0/k1 here,
                # push k2/k3 into pipeline filler tasks
                sk1 = emit_qk_strip(2, 0, p_pro, "prostrip")
                sk2 = emit_qk_strip(3, 0, p_pro, "prostrip")
                emit_rope(sk1, sk2, k1c[0], k2c[0], 0, rope_t)
                sq1 = emit_qk_strip(0, 0, p_pro, "prostrip")
                sq2 = emit_qk_strip(1, 0, p_pro, "prostrip")
                emit_rope(sq1, sq2, q1c[0], q2c[0], 0, rope_t)
                sk1 = emit_qk_strip(2, 1, p_pro, "prostrip")
                sk2 = emit_qk_strip(3, 1, p_pro, "prostrip")
                emit_rope(sk1, sk2, k1c[1], k2c[1], 1, rope_t)
                for nb in range(4):
                    emit_v(nb, p_prov, "vpro")

            # ---- phase 2: flat attention pipeline + gathered projection ----
            # Remaining V j-groups and q-chunks 1-3 are injected as filler
            # tasks into the early pipeline slots.
            with tc.tile_pool(name="e", bufs=2) as e_pool, \
                 tc.tile_pool(name="red", bufs=2) as red, \
                 tc.tile_pool(name="comb", bufs=2) as comb, \
                 tc.tile_pool(name="agst", bufs=2) as agst, \
                 tc.tile_pool(name="prhs", bufs=4) as prhs, \
                 tc.tile_pool(name="yst", bufs=1) as yst, \
                 tc.tile_pool(name="p_s", bufs=2, space="PSUM") as p_s, \
                 tc.tile_pool(name="p_pv", bufs=4, space="PSUM") as p_pv:

                def emit_allgather(ic):
                    if collective:
                        nc.gpsimd.collective_compute(
                            "AllGather",
                            AluOpType.bypass,
                            replica_groups=[list(range(NCORES))],
                            ins=[ag_in[ic][:, :]],
                            outs=[ag_out[ic][:, :]],
                        )
                    else:
                        # timing-only stand-in (single-core TimelineSim)
                        nc.sync.dma_start(ag_out[ic][: HPC * DV, :], ag_in[ic][:, :])

                proj_state = {}

                def emit_proj_dma(ic):
                    rchs = []
                    for g in range(4):
                        rch = prhs.tile(
                            [P, 4, CH], bf, tag="rch", name=f"rch{ic}_{g}", bufs=4
                        )
                        nc.sync.dma_start(
                            rch,
                            ag_out[ic][g * 4 * P : (g + 1) * 4 * P, :].rearrange(
                                "(t p) n -> p t n", p=P
                            ),
                        )
                        rchs.append(rch)
                    proj_state[ic] = rchs

                def emit_proj_mm(ic):
                    py = p_pv.tile([P, CH], f32, tag="pv", name=f"py{ic}")
                    rchs = proj_state.pop(ic)
                    for g in range(4):
                        for t in range(4):
                            kt = g * 4 + t
                            nc.tensor.matmul(
                                py, lhsT=wpj_sb[kt], rhs=rchs[g][:, t, :],
                                start=(kt == 0), stop=(kt == JT - 1),
                            )
                    ysb = yst.tile([P, CH], f32, tag="ysb", name=f"ysb{ic}")
                    nc.scalar.copy(ysb, py)
                    nc.sync.dma_start(yT.ap()[:, ic * CH : (ic + 1) * CH], ysb)

                py3 = [None]

                def emit_proj3_head(hh):
                    # ag_out3h[hh] rows r*128 : (r+1)*128 hold head (2r + hh),
                    # i.e. global kt = 2r + hh
                    if py3[0] is None:
                        py3[0] = p_pv.tile([P, CH], f32, tag="pv", name="py3")
                    py = py3[0]
                    rchs = []
                    for g2 in range(2):
                        rch = prhs.tile(
                            [P, 4, CH], bf, tag="rch", name=f"rch3{hh}_{g2}", bufs=4
                        )
                        nc.sync.dma_start(
                            rch,
                            ag_out3h[hh][g2 * 4 * P : (g2 + 1) * 4 * P, :].rearrange(
                                "(t p) n -> p t n", p=P
                            ),
                        )
                        rchs.append(rch)
                    for g2 in range(2):
                        for t in range(4):
                            r = g2 * 4 + t
                            kt = 2 * r + hh
                            nc.tensor.matmul(
                                py, lhsT=wpj_sb[kt], rhs=rchs[g2][:, t, :],
                                start=(hh == 0 and r == 0),
                                stop=(hh == 1 and r == 7),
                                skip_group_check=True,
                            )
                    if hh == 1:
                        ysb = yst.tile([P, CH], f32, tag="ysb", name="ysb3")
                        nc.scalar.copy(ysb, py)
                        nc.sync.dma_start(
                            yT.ap()[:, (NCH - 1) * CH : NCH * CH], ysb
                        )

                UNITS = [(ic, hh) for ic in range(NCH) for hh in range(HPC)]
                NU = len(UNITS)
                NPAIR = JT // 2
                st = [None] * NU

                def unit_alloc(u):
                    st[u] = dict(
                        e1=e_pool.tile([P, JT, CH], bf, tag="e1", name=f"e1_{u}"),
                        e2=e_pool.tile([P, JT, CH], bf, tag="e2", name=f"e2_{u}"),
                        pv1=p_pv.tile([P, CH], f32, tag="pv", name=f"pv1_{u}"),
                        pv2=p_pv.tile([P, CH], f32, tag="pv", name=f"pv2_{u}"),
                        acc1=red.tile([P, CH], f32, tag="acc1", name=f"acc1_{u}"),
                        # bf16: walrus rejects mixed-dtype TensorTensor on
                        # GPSIMD; the 128-partial average washes out rounding
                        acc2=red.tile([P, CH], bf, tag="acc2", name=f"acc2_{u}"),
                    )

                def emit_s(u, p):
                    ic, hh = UNITS[u]
                    hs = slice(HD * hh, HD * (hh + 1))
                    jb = 2 * p
                    c, o0, o1 = jb // 4, (jb % 4) * P, (jb % 4 + 1) * P
                    e1, e2 = st[u]["e1"], st[u]["e2"]
                    s1 = p_s.tile([P, 2, CH], f32, tag="s", name="s1t")
                    nc.tensor.matmul(
                        s1[:, 0, :], lhsT=k1c[c][hs, o0 : o0 + P],
                        rhs=q1c[ic][hs, :], start=True, stop=True,
                    )
                    nc.tensor.matmul(
                        s1[:, 1, :], lhsT=k1c[c][hs, o1 : o1 + P],
                        rhs=q1c[ic][hs, :], start=True, stop=True,
                    )
                    nc.scalar.activation(e1[:, jb : jb + 2, :], s1, Exp, scale=SCALE)
                    s2 = p_s.tile([P, 2, CH], f32, tag="s", name="s2t")
                    nc.tensor.matmul(
                        s2[:, 0, :], lhsT=k2c[c][hs, o0 : o0 + P],
                        rhs=q2c[ic][hs, :], start=True, stop=True,
                    )
                    nc.tensor.matmul(
                        s2[:, 1, :], lhsT=k2c[c][hs, o1 : o1 + P],
                        rhs=q2c[ic][hs, :], start=True, stop=True,
                    )
                    nc.scalar.activation(e2[:, jb : jb + 2, :], s2, Exp, scale=SCALE)

                def emit_acc(u, p):
                    # softmax denominators: E1 strips summed on DVE, E2 split
                    # DVE/GPSIMD (Pool adds are ~1.9x slower than DVE), one
                    # pair behind the exps
                    e1, e2 = st[u]["e1"], st[u]["e2"]
                    acc1, acc2 = st[u]["acc1"], st[u]["acc2"]
                    jb = 2 * p
                    e2_eng = nc.vector if (p in (0, 6, 7) or u == NU - 1) else nc.gpsimd
                    if p == 0:
                        nc.vector.tensor_tensor(
                            acc1, e1[:, 0, :], e1[:, 1, :], AluOpType.add
                        )
                        e2_eng.tensor_tensor(
                            acc2, e2[:, 0, :], e2[:, 1, :], AluOpType.add
                        )
                    else:
                        for j in (jb, jb + 1):
                            nc.vector.tensor_tensor(
                                acc1, acc1, e1[:, j, :], AluOpType.add
                            )
                            e2_eng.tensor_tensor(
                                acc2, acc2, e2[:, j, :], AluOpType.add
                            )

                def finish_acc(u):
                    acc1, acc2 = st[u]["acc1"], st[u]["acc2"]
                    s1bc = red.tile([P, CH], f32, tag="s1bc", name=f"s1bc{u}", bufs=2)
                    nc.gpsimd.partition_all_reduce(
                        s1bc, acc1, 128, bass_isa.ReduceOp.add
                    )
                    s2bc = red.tile([P, CH], f32, tag="s2bc", name=f"s2bc{u}", bufs=2)
                    nc.gpsimd.partition_all_reduce(
                        s2bc, acc2, 128, bass_isa.ReduceOp.add
                    )
                    st[u]["s1bc"], st[u]["s2bc"] = s1bc, s2bc

                def finish_recip(u):
                    if "r1" in st[u]:
                        return
                    # one slot after the partition reduce so the reciprocal
                    # doesn't head-of-line-block DVE while Pool finishes
                    r1 = red.tile([P, CH], f32, tag="r1", name=f"r1_{u}", bufs=1)
                    nc.vector.reciprocal(r1, st[u]["s1bc"])
                    r2 = red.tile([P, CH], f32, tag="r2", name=f"r2_{u}", bufs=1)
                    nc.vector.reciprocal(r2, st[u]["s2bc"])
                    st[u]["r1"], st[u]["r2"] = r1, r2

                def emit_pv(u, p):
                    ic, hh = UNITS[u]
                    e1, e2 = st[u]["e1"], st[u]["e2"]
                    pv1, pv2 = st[u]["pv1"], st[u]["pv2"]
                    for j in (2 * p, 2 * p + 1):
                        vt = vsb[hh][j // 4][:, j % 4, :]
                        nc.tensor.matmul(
                            pv1, lhsT=vt, rhs=e1[:, j, :],
                            start=(j == 0), stop=(j == JT - 1),
                            skip_group_check=True,
                        )
                        nc.tensor.matmul(
                            pv2, lhsT=vt, rhs=e2[:, j, :],
                            start=(j == 0), stop=(j == JT - 1),
                            skip_group_check=True,
                        )

                def finish_unit(u):
                    ic, hh = UNITS[u]
                    pv1, pv2 = st[u]["pv1"], st[u]["pv2"]
                    r1, r2 = st[u]["r1"], st[u]["r2"]
                    t1 = comb.tile([P, CH], f32, tag="t1", name=f"t1_{u}")
                    nc.vector.tensor_tensor(t1, pv1, r1, AluOpType.mult)
                    t2 = comb.tile([P, CH], f32, tag="t2", name=f"t2_{u}")
                    nc.vector.scalar_tensor_tensor(
                        t2, pv2, nlam_bc, r2, AluOpType.mult, AluOpType.mult
                    )
                    oc = agst.tile([P, CH], bf, tag="oc", name=f"oc{u}")
                    nc.vector.tensor_tensor(oc, t1, t2, AluOpType.add)
                    nc.sync.dma_start(ag_in[ic][hh * P : (hh + 1) * P, :], oc)
                    if ic == NCH - 1:
                        # per-head gather: h=0 fires a whole unit earlier
                        if collective:
                            nc.gpsimd.collective_compute(
                                "AllGather",
                                AluOpType.bypass,
                                replica_groups=[list(range(NCORES))],
                                ins=[ag_in[ic][hh * P : (hh + 1) * P, :]],
                                outs=[ag_out3h[hh][:, :]],
                            )
                        else:
                            nc.sync.dma_start(
                                ag_out3h[hh][:P, :], ag_in[ic][hh * P : (hh + 1) * P, :]
                            )
                    elif hh == HPC - 1:
                        emit_allgather(ic)

                # filler tasks for early slots: 2 V j-blocks per slot, then
                # the remaining q-chunk projections
                def v_task(nbs):
                    emit_v(nbs, p_pv, "pv")
                    emit_v(nbs + 1, p_pv, "pv")

                def q_task(nci):
                    sq1 = emit_qk_strip(0, nci, p_s, "s")
                    sq2 = emit_qk_strip(1, nci, p_s, "s")
                    emit_rope(sq1, sq2, q1c[nci], q2c[nci], nci, rope_t)

                def k_task(nci):
                    sk1 = emit_qk_strip(2, nci, p_s, "s")
                    sk2 = emit_qk_strip(3, nci, p_s, "s")
                    emit_rope(sk1, sk2, k1c[nci], k2c[nci], nci, rope_t)

                tasks = [
                    lambda: k_task(2),
                    lambda: v_task(4),
                    lambda: v_task(6),
                    lambda: k_task(3),
                    lambda: v_task(8),
                    lambda: v_task(10),
                    lambda: v_task(12),
                    lambda: v_task(14),
                    lambda: q_task(1),
                    lambda: q_task(2),
                    lambda: q_task(3),
                ]

                ACC_LAG, PV_LAG = 1, 4
                total = NU * NPAIR
                proj_at = {(2 * ic + 3): ic for ic in range(NCH - 1)}
                # final-chunk proj handled via emit_proj3_head
                LAST_PV_LAG = 2  # shrink the exposed epilogue of the last unit

                def pv_lag(pair):
                    return LAST_PV_LAG if pair >= (NU - 1) * NPAIR else PV_LAG

                pv_next = 0
                for g in range(total + PV_LAG + 1):
                    if tasks and g < 11:
                        tasks.pop(0)()
                    if g < total:
                        u, p = divmod(g, NPAIR)
                        if p == 0:
                            unit_alloc(u)
                            if u in proj_at:
                                emit_proj_dma(proj_at[u])
                        if p == 5 and u in proj_at:
                            emit_proj_mm(proj_at[u])
                        emit_s(u, p)
                    ga = g - ACC_LAG
                    if 0 <= ga < total:
                        u, p = divmod(ga, NPAIR)
                        emit_acc(u, p)
                        if p == NPAIR - 1:
                            finish_acc(u)
                    gr = g - ACC_LAG - 1
                    if 0 <= gr < total:
                        u, p = divmod(gr, NPAIR)
                        if p == NPAIR - 1:
                            finish_recip(u)
                    while pv_next < total and pv_next + pv_lag(pv_next) <= g:
                        u, p = divmod(pv_next, NPAIR)
                        emit_pv(u, p)
                        if p == NPAIR - 1:
                            if u == NU - 1:
                                finish_recip(u)
                            finish_unit(u)
                        pv_next += 1
                    if g == total - 3:
                        emit_proj3_head(0)
                emit_proj3_head(1)
            rope_cm.__exit__(None, None, None)

    nc.compile()
    return nc


def _get_prog():
    global _PROG
    if _PROG is None:
        _PROG = _build_program()
    return _PROG


def _prep_in_maps(x, W_qkv, W_proj, lambda_q1, lambda_q2, lambda_k1, lambda_k2):
    x = np.asarray(x, np.float32).reshape(N, C)
    W_qkv = np.asarray(W_qkv, np.float32)
    W_proj = np.asarray(W_proj, np.float32)

    xT = np.ascontiguousarray(x.T).astype(BF16)

    inv_freq = 1.0 / (10000.0 ** (np.arange(0, DV, 2, dtype=np.float32) / DV))
    freqs = np.arange(N, dtype=np.float32)[:, None] * inv_freq[None, :]  # [N, 64]
    cos = np.cos(freqs).astype(np.float32).T  # [64, N]
    sin = np.sin(freqs).astype(np.float32).T
    cosT = np.concatenate([cos, cos], axis=0).astype(BF16)  # [128, N]
    sinT = np.concatenate([sin, sin], axis=0).astype(BF16)

    lam_init = 0.8 - 0.6 * float(np.exp(-0.3 * 0.0))
    lam = (
        float(np.exp(np.sum(lambda_q1.astype(np.float32) * lambda_k1.astype(np.float32))))
        - float(np.exp(np.sum(lambda_q2.astype(np.float32) * lambda_k2.astype(np.float32))))
        + lam_init
    )
    nlam = np.full((1, 1), -lam, dtype=np.float32)

    W_projT = np.ascontiguousarray(W_proj.T)  # [2C, C]

    in_maps = []
    for r in range(NCORES):
        hA, hB = 2 * r, 2 * r + 1
        idx_qk = np.concatenate(
            [
                np.arange(g * C + h * HD, g * C + (h + 1) * HD)
                for g in range(4)
                for h in (hA, hB)
            ]
        )
        idx_v = np.concatenate(
            [
                np.arange(g * C + h * HD, g * C + (h + 1) * HD)
                for h in (hA, hB)
                for g in (4, 5)
            ]
        )
        wqk_r = np.ascontiguousarray(W_qkv[idx_qk, :].T).astype(BF16)  # [C, 512]
        wv_r = np.ascontiguousarray(W_qkv[idx_v, :].T).astype(BF16)  # [C, 256]
        wpj_r = np.ascontiguousarray(W_projT[:, r * P : (r + 1) * P]).astype(BF16)
        in_maps.append(
            dict(
                xT=xT,
                wqk=wqk_r,
                wv=wv_r,
                wpj=wpj_r,
                cosT=cosT,
                sinT=sinT,
                nlam=nlam,
            )
        )
    return in_maps


LAST_EXEC_TIME_NS = None


def kernel(x, W_qkv, W_proj, lambda_q1, lambda_q2, lambda_k1, lambda_k2):
    global LAST_EXEC_TIME_NS
    nc = _get_prog()
    in_maps = _prep_in_maps(
        x, W_qkv, W_proj, lambda_q1, lambda_q2, lambda_k1, lambda_k2
    )
    res = run_bass_kernel_spmd(nc, in_maps, core_ids=list(range(NCORES)))
    LAST_EXEC_TIME_NS = res.exec_time_ns
    yT_full = np.concatenate([res.results[r]["yT"] for r in range(NCORES)], axis=0)
    return np.ascontiguousarray(yT_full.T).reshape(B, N, C).astype(np.float32)
